# revision 26
# baseline (speedup 1.0000x reference)
"""Trainium2 Bass kernel for nn_AttentionModel (additive attention + masked softmax).

Computes, for full inputs (B=64, L=4096, D=512, OUT=256):
    para_lin = para_encode_state @ W_para.T          [B, L, OUT]
    q_lin    = query @ W_query.T + b_query           [B, OUT]
    e        = tanh(para_lin + q_lin[:,None,:]) . attn_vec   [B, L]
    attn     = softmax(e) * mask;  out = attn / sum(attn)  (guarded)

Strategy: data-parallel over B across 8 NeuronCores (8 batches/core).
Device-side per core: fp16 matmuls on the PE (inputs cast during the DMA
load), fp32 PSUM accumulation, tanh+bias fused on ScalarE, e-reduction as a
second matmul with one-hot-batch attn_vec columns, masked softmax tail
(softmax's Z cancels against the renormalization and is never computed).

Notes: built on bacc.Bacc (nc.compile() runs generate_event_semaphores,
which legalizes the 1-wait-per-instruction hardware constraint). The whole
l-block is transposed by a single xbar DMA into a folded [d, (lt dc), j]
layout that the matmuls read back with a strided access pattern.
"""

import os
import sys

for _p in ("/opt/trn_rl_repo", "/root/.axon_site/_ro/trn_rl_repo"):
    if os.path.isdir(_p) and _p not in sys.path:
        sys.path.insert(0, _p)

import numpy as np

import concourse.bacc as bacc
import concourse.mybir as mybir
from concourse import tile
from concourse.bass_utils import run_bass_kernel_spmd

# Problem shape (hardcoded per contract)
B, L, DIN, OUT = 64, 4096, 512, 256
NCORES = 8
BPC = B // NCORES          # batches per core
LBLK = 2048                # l-block processed per inner step
NLB = L // LBLK            # 2 l-blocks
LT = LBLK // 128           # 16 [128, DIN] sub-tiles per l-block
DC = DIN // 128            # 4 contraction chunks
OC = OUT // 128            # 2 output-partition chunks
NH = LBLK // 512           # 4 512-wide psum chunks per l-block

FP16 = mybir.dt.float16
F32 = mybir.dt.float32

_NC_CACHE = {}


def _build_nc():
    nc = bacc.Bacc("TRN2", target_bir_lowering=False)
    para = nc.declare_dram_parameter("para", [BPC, L, DIN], F32, isOutput=False)
    wt = nc.declare_dram_parameter("wt", [DIN, OUT], FP16, isOutput=False)
    qlin = nc.declare_dram_parameter("qlin", [128, OC, BPC], F32, isOutput=False)
    av8 = nc.declare_dram_parameter("av8", [128, OC, BPC, BPC], FP16, isOutput=False)
    maskf = nc.declare_dram_parameter("maskf", [BPC, L], F32, isOutput=False)
    out_d = nc.declare_dram_parameter("out", [BPC, L], F32, isOutput=True)

    with tile.TileContext(nc) as tc:
        with (
            tc.tile_pool(name="const", bufs=1) as cpool,
            tc.tile_pool(name="a", bufs=3) as apool,
            tc.tile_pool(name="t", bufs=2) as tpool,
            tc.tile_pool(name="th", bufs=2) as thpool,
            tc.tile_pool(name="eb", bufs=1) as ebpool,
            tc.tile_pool(name="mm", bufs=2, space="PSUM") as mmpool,
            tc.tile_pool(name="eps", bufs=1, space="PSUM") as epool,
        ):
            # one-time loads (weights / per-batch vectors / mask)
            WT = cpool.tile([128, DC, OUT], FP16)
            nc.sync.dma_start(WT[:], wt.rearrange("(dc p) o -> p dc o", p=128))
            QL = cpool.tile([128, OC, BPC], F32)
            nc.sync.dma_start(QL[:], qlin[:])
            AV = cpool.tile([128, OC, BPC, BPC], FP16)
            nc.sync.dma_start(AV[:], av8[:])
            MS = cpool.tile([BPC, L], F32)
            nc.sync.dma_start(MS[:], maskf[:])

            EB = ebpool.tile([BPC, L], F32)

            for lb in range(NLB):
                EP = epool.tile([BPC, LBLK], F32)
                for b in range(BPC):
                    # cast-load: fp32 HBM -> fp16 SBUF [128, LT, DIN] (SWDGE)
                    A = apool.tile([128, LT, DIN], FP16)
                    nc.gpsimd.dma_start(
                        out=A[:],
                        in_=para[b, lb * LBLK : (lb + 1) * LBLK, :].rearrange(
                            "(lt p) d -> p lt d", p=128
                        ),
                    )
                    # single xbar transpose to [d, (lt dc), j] for the whole
                    # block: out[p, lt*DC+dc, j] = A[j, lt, dc*128+p]
                    T = tpool.tile([128, LT, DC, 128], FP16)
                    nc.sync.dma_start(
                        out=T[:].rearrange("p lt dc j -> p (lt dc) j"),
                        in_=A[:].rearrange("p lt d -> p (lt d)"),
                        transpose=True,
                    )
                    # para_lin matmuls + fused tanh(psum + q_lin)
                    TH = thpool.tile([128, OC, LBLK], FP16)
                    for oc in range(OC):
                        for nhg in range(NH // 2):
                            PM = mmpool.tile([128, 1024], F32)
                            for nh2 in range(2):
                                nh = nhg * 2 + nh2
                                for dc in range(DC):
                                    nc.tensor.matmul(
                                        PM[:, nh2 * 512 : (nh2 + 1) * 512],
                                        WT[:, dc, oc * 128 : (oc + 1) * 128],
                                        T[:, nh * 4 : nh * 4 + 4, dc, :],
                                        start=(dc == 0),
                                        stop=(dc == DC - 1),
                                    )
                            nc.scalar.activation(
                                TH[:, oc, nhg * 1024 : (nhg + 1) * 1024],
                                PM[:],
                                mybir.ActivationFunctionType.Tanh,
                                bias=QL[:, oc, b : b + 1],
                                scale=1.0,
                            )
                    # e-reduction: one-hot-batch attn_vec columns; all 8 batches
                    # accumulate into one PSUM [BPC, LBLK]
                    for nh in range(NH):
                        for oc in range(OC):
                            nc.tensor.matmul(
                                EP[:, nh * 512 : (nh + 1) * 512],
                                AV[:, oc, b, :],
                                TH[:, oc, nh * 512 : (nh + 1) * 512],
                                start=(b == 0 and oc == 0),
                                stop=(b == BPC - 1 and oc == OC - 1),
                            )
                nc.vector.tensor_copy(EB[:, lb * LBLK : (lb + 1) * LBLK], EP[:])

            # tail: masked softmax with cancelled Z
            MX = ebpool.tile([BPC, 1], F32)
            nc.vector.reduce_max(MX[:], EB[:], axis=mybir.AxisListType.X)
            NMX = ebpool.tile([BPC, 1], F32)
            nc.vector.tensor_scalar_mul(NMX[:], MX[:], -1.0)
            EX = ebpool.tile([BPC, L], F32)
            nc.scalar.activation(
                EX[:], EB[:], mybir.ActivationFunctionType.Exp, bias=NMX[:], scale=1.0
            )
            nc.vector.tensor_mul(EX[:], EX[:], MS[:])
            S = ebpool.tile([BPC, 1], F32)
            nc.vector.reduce_sum(S[:], EX[:], axis=mybir.AxisListType.X)
            S2 = ebpool.tile([BPC, 1], F32)
            nc.vector.tensor_scalar_max(S2[:], S[:], 1e-30)
            R = ebpool.tile([BPC, 1], F32)
            nc.vector.reciprocal(R[:], S2[:])
            nc.vector.tensor_scalar_mul(EX[:], EX[:], R[:])
            nc.sync.dma_start(out_d[:], EX[:])
    nc.compile()
    return nc


def get_nc():
    if "nc" not in _NC_CACHE:
        _NC_CACHE["nc"] = _build_nc()
    return _NC_CACHE["nc"]


def _host_prep(para, query, mask, w_para, w_query, b_query, attn_vec):
    para = np.ascontiguousarray(np.asarray(para, dtype=np.float32))
    query = np.asarray(query, dtype=np.float32)
    mask = np.asarray(mask)
    w_para = np.asarray(w_para, dtype=np.float32)
    w_query = np.asarray(w_query, dtype=np.float32)
    b_query = np.asarray(b_query, dtype=np.float32)
    attn_vec = np.asarray(attn_vec, dtype=np.float32)

    wt = np.ascontiguousarray(w_para.T).astype(np.float16)          # [DIN, OUT]
    qlin = query @ w_query.T + b_query                              # [B, OUT] fp32
    qlt = np.ascontiguousarray(
        qlin.reshape(NCORES, BPC, OC, 128).transpose(0, 3, 2, 1)
    )                                                               # [NCORES,128,OC,BPC]
    av_pc = attn_vec.reshape(OC, 128).T                             # [128, OC]
    av8 = np.einsum("po,bj->pobj", av_pc, np.eye(BPC, dtype=np.float32))
    av8 = np.ascontiguousarray(av8).astype(np.float16)              # [128, OC, BPC, BPC]
    maskf = mask.astype(np.float32)                                 # [B, L]

    in_maps = []
    for c in range(NCORES):
        in_maps.append(
            {
                "para": np.ascontiguousarray(para[c * BPC : (c + 1) * BPC]),
                "wt": wt,
                "qlin": np.ascontiguousarray(qlt[c]),
                "av8": av8,
                "maskf": np.ascontiguousarray(maskf[c * BPC : (c + 1) * BPC]),
            }
        )
    return in_maps


def run(inputs, **spmd_kwargs):
    """Run on hardware; returns (out [B, L] fp32, BassKernelResults)."""
    in_maps = _host_prep(
        inputs["para_encode_state"],
        inputs["query"],
        inputs["enc_padding_mask"],
        inputs["W_para"],
        inputs["W_query"],
        inputs["b_query"],
        inputs["attn_vec"],
    )
    res = run_bass_kernel_spmd(
        get_nc(), in_maps, core_ids=list(range(NCORES)), **spmd_kwargs
    )
    out = np.concatenate([r["out"] for r in res.results], axis=0)
    return out, res


def kernel(**inputs) -> np.ndarray:
    out, _ = run(inputs)
    return out


if __name__ == "__main__":
    rng = np.random.default_rng(0)
    demo = {
        "para_encode_state": rng.standard_normal((B, L, DIN), dtype=np.float32),
        "query": rng.standard_normal((B, DIN), dtype=np.float32),
        "enc_padding_mask": rng.integers(0, 2, (B, L)).astype(np.int32),
        "W_para": (rng.standard_normal((OUT, DIN), dtype=np.float32) / np.sqrt(DIN)),
        "W_query": (rng.standard_normal((OUT, DIN), dtype=np.float32) / np.sqrt(DIN)),
        "b_query": np.zeros(OUT, dtype=np.float32),
        "attn_vec": rng.standard_normal(OUT, dtype=np.float32),
    }
    o = kernel(**demo)
    print("out", o.shape, o.dtype, float(o.sum()))
